# revision 32
# baseline (speedup 1.0000x reference)
"""CPCA-weighted loss kernel for 8 Trainium2 NeuronCores (fp8 pipeline).

Sharding: data-parallel over the env dim n (8 envs -> 1 env/core, params
replicated).  Each core runs the k=16-step GRU over its 256 sequences, the
two-layer classifier for pos/neg logits, softplus + weighted-mask reduce,
and returns per-core partial sums; the host combines them into the scalar
loss.

v2 restructurings vs the bf16 baseline (validated to 7e-5 rel in numpy):
  * All big matmuls in fp8-e4m3 with weights pre-scaled by 32; DoubleRow
    perf mode contracts 256 rows/pass (2x-4x PE throughput).  The 1/32
    descale rides the activation `scale` / stt scalar slots for free.
  * r/z gates keep the rank-8 "onehot" matmul (gi + b_hh into PSUM, fp8);
    the n-gate's b_hh uses the t1 stt scalar slot instead.
  * sigmoid(r) and sigmoid(z) merged into one wide ACT op per half-step
    reading two adjacent PSUM banks.
  * GRU elementwise chain expressed as scalar_tensor_tensor ops (4x DVE
    mode when all operands are bf16 SBUF); h kept in bf16 + fp8 copies
    (fp8 cast on the otherwise-idle Pool/GpSimd engine).
  * pos/neg classifier heads stacked on partitions 0-31/32-63: one
    qp matmul, one pre-stt, one relu per step for both heads.
"""

import numpy as np
import ml_dtypes

import concourse.bass as bass
import concourse.mybir as mybir
import concourse.tile as tile
from concourse import bacc
from concourse.bass_utils import run_bass_kernel_spmd

BF16 = mybir.dt.bfloat16
FP8 = mybir.dt.float8e4
F32 = mybir.dt.float32
AF = mybir.ActivationFunctionType
ALU = mybir.AluOpType
DR = mybir.MatmulPerfMode.DoubleRow

T, N, H, K, A = 256, 8, 512, 16, 4
NUM_ACTIONS = 6
P_SUB = 0.1
LOSS_FACTOR = 0.1
WEIGHT = np.array([5, 4, 3, 3, 2, 2, 2, 2, 1, 1, 1, 1, 1, 1, 1, 1], dtype=np.float32)

NCORES = 8
B = T * N // NCORES            # 256 sequences per core
BC = B // 128                  # 2 partition chunks of the batch
HC = H // 128                  # 4 partition chunks of the hidden dim
G = 3 * H                      # 1536 gate dim
PADW = T + K - 1               # 271 padded action-sequence length
S = 32.0                       # fp8 weight scale
SI = float(1.0 / S)

NP_FP8 = ml_dtypes.float8_e4m3
NP_BF16 = ml_dtypes.bfloat16

_NC_CACHE = {}


def _build_bass(zero_bhn=True):
    """Build the per-core Bass program (identical on all 8 cores)."""
    key = ("nc", zero_bhn)
    if key in _NC_CACHE:
        return _NC_CACHE[key]

    nc = bacc.Bacc("TRN2", target_bir_lowering=False, debug=False)

    # --- DRAM I/O ------------------------------------------------------
    # wt8: [128, HC, G] fp8 = (S*W_hh).T chunked; gate cols 0..511 r,
    # 512..1023 z, 1024..1535 n.
    d_wt = nc.dram_tensor("wt8", [128, HC, G], FP8, kind="ExternalInput")
    # gaug8: [40, H] fp8: rows 0-7 = S*(gi_r + b_hh_r) table (8 actions),
    # rows 32-39 = z ditto.  onehot8: [40, PADW] fp8 one-hot action seq at
    # the same partition strips.
    d_gaug = nc.dram_tensor("gaug8", [40, H], FP8, kind="ExternalInput")
    d_onehot = nc.dram_tensor("onehot8", [40, PADW], FP8, kind="ExternalInput")
    d_gin = nc.dram_tensor("gin", [128, HC, PADW], BF16, kind="ExternalInput")
    d_bel8 = nc.dram_tensor("belief8", [128, HC, B], FP8, kind="ExternalInput")
    d_belb = nc.dram_tensor("beliefb", [128, HC, B], BF16, kind="ExternalInput")
    d_vis8 = nc.dram_tensor("vis8", [128, HC, B], FP8, kind="ExternalInput")
    d_neg8 = nc.dram_tensor("neg8", [128, HC, B], FP8, kind="ExternalInput")
    # w1a8: [128, HC, 32] fp8 (e-part, S*W1a.T); w1q8: [128, HC, 64] fp8
    # (q-part S*W1b.T duplicated to cols 32-63 for the stacked heads)
    d_w1a = nc.dram_tensor("w1a8", [128, HC, 32], FP8, kind="ExternalInput")
    d_w1q = nc.dram_tensor("w1q8", [128, HC, 64], FP8, kind="ExternalInput")
    # w2s: [64, 1] bf16: rows 0-31 = -W2 (pos sign), rows 32-63 = +W2
    d_w2 = nc.dram_tensor("w2s", [64, 1], BF16, kind="ExternalInput")
    d_b1 = nc.dram_tensor("b1d", [64, 1], F32, kind="ExternalInput")
    d_bhn = nc.dram_tensor("bhn32", [128, HC], F32, kind="ExternalInput")
    d_b2pat = nc.dram_tensor("b2pat", [128, 4, K], F32, kind="ExternalInput")
    d_wm = nc.dram_tensor("wmask", [128, 4, K], F32, kind="ExternalInput")
    d_out = nc.dram_tensor("partials", [128, 4], F32, kind="ExternalOutput")

    with tile.TileContext(nc) as tc:
        with (
            tc.tile_pool(name="const", bufs=1) as const,
            tc.tile_pool(name="hbp", bufs=1) as hbp,
            tc.tile_pool(name="h8p", bufs=1) as h8p,
            tc.tile_pool(name="gates", bufs=1) as gates,
            tc.tile_pool(name="cls", bufs=1) as cls,
            tc.tile_pool(name="rzp", bufs=1, space="PSUM") as rzp,
            tc.tile_pool(name="npp", bufs=1, space="PSUM") as npp,
            tc.tile_pool(name="spsum", bufs=1, space="PSUM") as spsum,
            tc.tile_pool(name="lpsum", bufs=1, space="PSUM") as lpsum,
        ):
            # --- constants into SBUF (ordered by first use) -------------
            w1a = const.tile([128, HC, 32], FP8)
            nc.sync.dma_start(w1a[:], d_w1a[:])
            vis = const.tile([128, HC, B], FP8)
            nc.sync.dma_start(vis[:], d_vis8[:])
            neg = const.tile([128, HC, B], FP8)
            nc.gpsimd.dma_start(neg[:], d_neg8[:])
            b1 = const.tile([64, 1], F32)
            nc.sync.dma_start(b1[:], d_b1[:])
            # h kept as per-half tiles so the kt0 matmul wave of step m+1
            # can start as soon as half 0 of h(m) is cast (precise deps)
            h8_0 = [h8p.tile([128, 2, B], FP8, tag=f"h8_{t}", bufs=2,
                             name=f"h8i_{t}") for t in range(2)]
            for t in range(2):
                nc.sync.dma_start(h8_0[t][:], d_bel8[:, 2 * t:2 * t + 2, :])
            gaug = const.tile([40, H], FP8)
            nc.gpsimd.dma_start(gaug[:], d_gaug[:])
            onehot = const.tile([40, PADW], FP8)
            nc.gpsimd.dma_start(onehot[:], d_onehot[:])
            dq = [nc.sync, nc.gpsimd]
            wtj = const.tile([128, HC, G], FP8, name="wt8")
            for j in range(6):
                dq[j % 2].dma_start(
                    wtj[:, :, 256 * j:256 * (j + 1)],
                    d_wt[:, :, 256 * j:256 * (j + 1)])
            hb_0 = [hbp.tile([128, 2, B], BF16, tag=f"hb_{t}", bufs=2,
                             name=f"hbi_{t}") for t in range(2)]
            for t in range(2):
                nc.sync.dma_start(hb_0[t][:], d_belb[:, 2 * t:2 * t + 2, :])
            gin = const.tile([128, HC, PADW], BF16)
            nc.gpsimd.dma_start(gin[:], d_gin[:])
            w1q = const.tile([128, HC, 64], FP8)
            nc.sync.dma_start(w1q[:], d_w1q[:])
            w2 = const.tile([64, 1], BF16)
            nc.sync.dma_start(w2[:], d_w2[:])
            bhn = const.tile([128, HC], F32)
            nc.sync.dma_start(bhn[:], d_bhn[:])
            b2pat = const.tile([128, 4, K], F32)
            nc.gpsimd.dma_start(b2pat[:], d_b2pat[:])
            wm = const.tile([128, 4, K], F32)
            nc.sync.dma_start(wm[:], d_wm[:])

            # --- V1/N1: e-part of classifier, heads stacked [64, B] ----
            # vpad[p, c] = (S*vis[c] @ W1a.T)/S + b1 for c<T, 0 for pads
            ps_vn = spsum.tile([64, B], F32, tag="s", bufs=1, name="ps_vn")
            for kt in range(2):
                nc.tensor.matmul(
                    ps_vn[0:32, :], w1a[:, 2 * kt:2 * kt + 2, :],
                    vis[:, 2 * kt:2 * kt + 2, :],
                    start=(kt == 0), stop=(kt == 1), perf_mode=DR,
                )
            # (DoubleRow + column tile_position fails the ISA check; the
            # neg head runs as 4 plain fp8 matmuls instead)
            for kk in range(HC):
                nc.tensor.matmul(
                    ps_vn[32:64, :], w1a[:, kk, :], neg[:, kk, :],
                    start=(kk == 0), stop=(kk == HC - 1),
                    tile_position=(0, 32),
                )
            # vpad kept at S-scale (descale folded into w2s host-side);
            # b1d is S*b1.  Pad region stays 0 (always masked out).
            vpad = cls.tile([64, PADW + 1], BF16, tag="vpad", name="vpad")
            nc.vector.memset(vpad[:, T:], 0.0)
            nc.scalar.activation(vpad[:, 0:T], ps_vn[:], AF.Copy)
            nc.vector.tensor_scalar_add(vpad[:, 0:T], vpad[:, 0:T],
                                        b1[:, 0:1])

            # logits live in one persistent PSUM bank:
            # [batch-part, (pos0,pos1,neg0,neg1), m], sign folded into w2
            logits = lpsum.tile([128, 4, K], F32, tag="l", bufs=1,
                                name="logits")

            h8 = h8_0
            hb = hb_0
            # classifier state carried one step behind the GRU
            prev_h1 = None
            prev_qp = None

            rcol = lambda j: bass.ts(j, 128)
            zcol = lambda j: bass.ts(HC + j, 128)
            ncol = lambda j: bass.ts(2 * HC + j, 128)

            def emit_logits(ph1, pm):
                for c in range(2):
                    for ch in range(BC):
                        nc.tensor.matmul(
                            logits[:, 2 * c + ch, pm:pm + 1],
                            ph1[32 * c:32 * c + 32, bass.ts(ch, 128)],
                            w2[32 * c:32 * c + 32, 0:1],
                            start=True, stop=True, tile_position=(32 * c, 0),
                        )

            def emit_cls_vector(pqp, pm):
                # pre kept at S-scale; w2s is pre-divided by S host-side.
                # Deprioritized: the scheduler slots these into DVE idle
                # time instead of ahead of the GRU chain ops (the logits
                # have a full step of slack).
                save_prio = tc.cur_priority
                tc.cur_priority = save_prio + 1_000_000
                pre = cls.tile([64, B], BF16, tag="pre", bufs=2,
                               name=f"pre{pm}")
                nc.vector.tensor_add(pre[:], pqp[:],
                                     vpad[:, pm + 1:pm + 1 + B])
                h1 = cls.tile([64, B], BF16, tag="h1", bufs=2,
                              name=f"h1{pm}")
                nc.vector.tensor_scalar_max(h1[:], pre[:], 0.0)
                tc.cur_priority = save_prio
                return h1

            for m in range(K):
                # ---- PE program for this step ------------------------
                # onehot wave (no h dependency), then the kt0 wave (only
                # needs half 0 of h(m-1)), qp of step m-1, the kt1 wave
                # (needs half 1), and finally logits of step m-1.  The
                # early waves keep the PE busy while half 1 of h(m-1) is
                # still being produced by the vector chain.
                # one PSUM tile per (gate, half): dependency tracking is
                # tile-granular, so sigmoid(r) must not share a tile with
                # the z accumulation groups (it would wait for them)
                rp = [rzp.tile([128, 2, B], F32, tag=f"rp{t}", bufs=1,
                               name=f"rp{m}_{t}") for t in range(2)]
                zp = [rzp.tile([128, 2, B], F32, tag=f"zp{t}", bufs=1,
                               name=f"zp{m}_{t}") for t in range(2)]
                nps = [npp.tile([128, 2, B], F32, tag=f"np{t}", bufs=1,
                                name=f"np{m}_{t}") for t in range(2)]
                for t in range(2):
                    for jj in range(2):
                        j = 2 * t + jj
                        nc.tensor.matmul(
                            rp[t][:, jj, :], gaug[0:8, rcol(j)],
                            onehot[0:8, m:m + B],
                            start=True, stop=False, tile_position=(0, 0),
                        )
                        nc.tensor.matmul(
                            zp[t][:, jj, :], gaug[32:40, rcol(j)],
                            onehot[32:40, m:m + B],
                            start=True, stop=False, tile_position=(32, 0),
                        )
                # group-complete order: r(h0), n(h0), r(h1), z(h0),
                # n(h1), z(h1) -- sigmoid(r, h0) and t1(h0) unblock after
                # 4/8 matmuls instead of after the whole 24-matmul train.
                # qp kt0 leads the train: it only needs h half 0 and
                # covers the latency until cast(h1) of the previous step
                # lands (needed by every kt1 matmul).
                def gate_mm(gate, t, jj, kt):
                    j = 2 * t + jj
                    ktile = slice(2 * kt, 2 * kt + 2)
                    if gate == "n":
                        nc.tensor.matmul(
                            nps[t][:, jj, :], wtj[:, ktile, ncol(j)],
                            h8[kt][:],
                            start=(kt == 0), stop=(kt == 1), perf_mode=DR,
                        )
                    else:
                        dst = rp[t] if gate == "r" else zp[t]
                        col = rcol(j) if gate == "r" else zcol(j)
                        nc.tensor.matmul(
                            dst[:, jj, :], wtj[:, ktile, col],
                            h8[kt][:],
                            start=False, stop=(kt == 1), perf_mode=DR,
                        )

                # Interleaved train: kt1 matmuls of a group follow its
                # kt0 pair as soon as possible (kt1 needs cast(h1) of the
                # previous step, which lands ~6 matmuls into the train),
                # so sigmoid(r,h0) fires at matmul #8 instead of #15 and
                # the h0 chain overlaps the rest of the train.
                train = [("r", 0, 0), ("n", 0, 0), ("r", 1, 0),
                         ("r", 0, 1),                       # -> sig(r,h0)
                         ("n", 0, 1),                       # -> t1(h0)
                         ("z", 0, 0), ("z", 0, 1),          # -> sig(z,h0)
                         ("n", 1, 0), ("qp", 0, 0), ("z", 1, 0),
                         ("r", 1, 1),                       # -> sig(r,h1)
                         ("n", 1, 1),                       # -> t1(h1)
                         ("z", 1, 1),                       # -> sig(z,h1)
                         ("qp", 1, 0)]
                for gate, t, kt in train:
                    if gate == "qp":
                        if prev_qp is not None:
                            nc.tensor.matmul(
                                prev_qp[:], w1q[:, 2 * t:2 * t + 2, :],
                                h8[t][:],
                                start=(t == 0), stop=(t == 1), perf_mode=DR,
                            )
                    else:
                        for jj in range(2):
                            gate_mm(gate, t, jj, kt)
                if prev_h1 is not None:
                    emit_logits(*prev_h1)

                # ---- ACT/DVE interleave (emission = producer order;
                # ACT queue lands as sr0, sz0, sr1, tanh0, sz1, tanh1) --
                rzbs = [gates.tile([128, 4, B], BF16, tag="rzb", bufs=2,
                                   name=f"rzb{m}_{t}") for t in range(2)]
                cands = [gates.tile([128, 2, B], BF16, tag="cand", bufs=2,
                                    name=f"c{m}_{t}") for t in range(2)]
                t2s = [gates.tile([128, 2, B], BF16, tag="t2", bufs=2,
                                  name=f"t2{m}_{t}") for t in range(2)]
                t1s = [gates.tile([128, 2, B], BF16, tag="t1", bufs=2,
                                  name=f"t1{m}_{t}") for t in range(2)]

                def emit_t1t2(t):
                    if zero_bhn:
                        # t1 = psum_n * r in one [512] tt (b_hh_n == 0)
                        nc.vector.tensor_mul(t1s[t][:], nps[t][:],
                                             rzbs[t][:, 0:2, :])
                    else:
                        for jj in range(2):
                            j = 2 * t + jj
                            nc.vector.scalar_tensor_tensor(
                                out=t1s[t][:, jj, :], in0=nps[t][:, jj, :],
                                scalar=bhn[:, j:j + 1], op0=ALU.add,
                                in1=rzbs[t][:, jj, :], op1=ALU.mult,
                            )
                    nc.vector.tensor_add(
                        t2s[t][:], t1s[t][:],
                        gin[:, 2 * t:2 * t + 2, m:m + B])

                h8n = [h8p.tile([128, 2, B], FP8, tag=f"h8_{t}", bufs=2,
                                name=f"h8_{m + 1}_{t}") for t in range(2)]
                hbn = [hbp.tile([128, 2, B], BF16, tag=f"hb_{t}", bufs=2,
                                name=f"hb_{m + 1}_{t}") for t in range(2)]

                def emit_tail(t, cast_on_act):
                    # d = h - cand ; e = z*d ; h' = cand + e  (tt, 2x)
                    d = gates.tile([128, 2, B], BF16, tag="d", bufs=2,
                                   name=f"d{m}_{t}")
                    nc.vector.tensor_sub(d[:], hb[t][:], cands[t][:])
                    e = gates.tile([128, 2, B], BF16, tag="e", bufs=2,
                                   name=f"e{m}_{t}")
                    nc.vector.tensor_mul(e[:], rzbs[t][:, 2:4, :], d[:])
                    nc.vector.tensor_add(hbn[t][:], cands[t][:], e[:])
                    if cast_on_act:
                        nc.scalar.activation(h8n[t][:], hbn[t][:], AF.Copy)
                    else:
                        nc.vector.tensor_copy(h8n[t][:], hbn[t][:])

                # Engine-queue orders (priority = emission):
                #   ACT: sr0, sr1, tanh0, sz0, sz1, tanh1, cast1
                #   DVE: t1/t2(h0), t1/t2(h1), tail0+cast0, tail1
                # cast0 stays on DVE (next op in queue after h'0, no
                # cross-engine hop on the critical cycle); cast1 rides
                # the ACT which has slack after tanh1.
                # Priority stratification: the h0 chain (loop-carried
                # critical path: sig(r,h0) -> t1 -> t2 -> tanh0 -> tail0
                # -> cast0 -> next kt0 wave) keeps base priority; all
                # h1-chain and secondary ACT work is pushed back so it
                # fills gaps instead of queueing ahead of the h0 chain.
                nc.scalar.activation(rzbs[0][:, 0:2, :], rp[0][:],
                                     AF.Sigmoid, scale=SI)
                emit_t1t2(0)
                nc.scalar.activation(cands[0][:], t2s[0][:], AF.Tanh,
                                     scale=SI)
                emit_tail(0, cast_on_act=False)
                save_prio = tc.cur_priority
                tc.cur_priority = save_prio + 200_000
                nc.scalar.activation(rzbs[1][:, 0:2, :], rp[1][:],
                                     AF.Sigmoid, scale=SI)
                nc.scalar.activation(rzbs[0][:, 2:4, :], zp[0][:],
                                     AF.Sigmoid, scale=SI)
                emit_t1t2(1)
                nc.scalar.activation(rzbs[1][:, 2:4, :], zp[1][:],
                                     AF.Sigmoid, scale=SI)
                nc.scalar.activation(cands[1][:], t2s[1][:], AF.Tanh,
                                     scale=SI)
                emit_tail(1, cast_on_act=True)
                tc.cur_priority = save_prio
                h8 = h8n
                hb = hbn
                # classifier vector work of m-1 last: logits have a full
                # step of slack, the gate chain does not
                if prev_qp is not None:
                    prev_h1 = (emit_cls_vector(prev_qp, m - 1), m - 1)

                # classifier psum for this step (filled next iteration)
                prev_qp = spsum.tile([64, B], F32, tag="s", bufs=1,
                                     name=f"qp{m}")

            # flush the last step's classifier
            nc.tensor.matmul(prev_qp[:], w1q[:, 0:2, :], h8[0][:],
                             start=True, stop=False, perf_mode=DR)
            nc.tensor.matmul(prev_qp[:], w1q[:, 2:4, :], h8[1][:],
                             start=False, stop=True, perf_mode=DR)
            h1f = emit_cls_vector(prev_qp, K - 1)
            emit_logits(h1f, K - 1)
            if prev_h1 is not None:
                emit_logits(*prev_h1)

            # --- softplus + mask-weight reduce ------------------------
            # logits hold s0 = -+(h1 @ W2.T); s = s0 + (-+b2);
            # softplus(s) = max(s,0) + ln(1 + exp(-|s|))
            partials = cls.tile([128, 4, 1], F32, tag="part", name="partials")
            s = cls.tile([128, 4, K], F32, tag="s_aff", name="s_aff")
            nc.vector.tensor_add(s[:], logits[:], b2pat[:])
            rl = cls.tile([128, 4, K], F32, tag="s_rl", name="s_rl")
            nc.vector.tensor_scalar_max(rl[:], s[:], 0.0)
            nab = cls.tile([128, 4, K], F32, tag="s_nab", name="s_nab")
            nc.vector.scalar_tensor_tensor(
                out=nab[:], in0=rl[:], scalar=-2.0, in1=s[:],
                op0=ALU.mult, op1=ALU.add,
            )
            ex = cls.tile([128, 4, K], F32, tag="s_ex", name="s_ex")
            nc.scalar.activation(ex[:], nab[:], AF.Exp)
            lg = cls.tile([128, 4, K], F32, tag="s_lg", name="s_lg")
            nc.scalar.activation(lg[:], ex[:], AF.Ln, bias=1.0)
            sp = cls.tile([128, 4, K], F32, tag="sp", name="sp")
            nc.vector.tensor_add(sp[:], rl[:], lg[:])
            tr = cls.tile([128, 4, K], F32, tag="tr", name="tr")
            nc.vector.tensor_mul(tr[:], sp[:], wm[:])
            nc.vector.reduce_sum(partials[:], tr[:], axis=mybir.AxisListType.X)
            nc.sync.dma_start(d_out[:], partials[:, :, 0])

    nc.compile()
    _NC_CACHE[key] = nc
    return nc


def _threefry_pair(k0, k1, x0, x1):
    """numpy port of jax's threefry2x32 primitive (verified bit-exact)."""
    x0 = x0.astype(np.uint32).copy()
    x1 = x1.astype(np.uint32).copy()
    ks0 = np.uint32(k0)
    ks1 = np.uint32(k1)
    ks2 = np.uint32(ks0 ^ ks1 ^ np.uint32(0x1BD11BDA))

    def rotl(x, d):
        return ((x << np.uint32(d)) | (x >> np.uint32(32 - d))).astype(np.uint32)

    rots = [[13, 15, 26, 6], [17, 29, 16, 24]]
    x0 = (x0 + ks0).astype(np.uint32)
    x1 = (x1 + ks1).astype(np.uint32)
    ks = [ks1, ks2, ks0]
    for i in range(5):
        for r in rots[i % 2]:
            x0 = (x0 + x1).astype(np.uint32)
            x1 = np.uint32(rotl(x1, r) ^ x0)
        x0 = (x0 + ks[i % 3]).astype(np.uint32)
        x1 = (x1 + ks[(i + 1) % 3] + np.uint32(i + 1)).astype(np.uint32)
    return x0, x1


def _uniform_lt(key, shape, thresh):
    """jax.random.uniform(key, shape) < thresh, threefry-partitionable spec."""
    num = int(np.prod(shape))
    b1, b2 = _threefry_pair(key[0], key[1], np.zeros(num, np.uint32),
                            np.arange(num, dtype=np.uint32))
    bits = b1 ^ b2
    fl = ((bits >> np.uint32(9)) | np.uint32(0x3F800000)).view(np.float32) \
        - np.float32(1.0)
    fl = np.maximum(fl, np.float32(0.0))
    return (fl < np.float32(thresh)).reshape(shape)


def _sub_masks():
    """The reference's input-independent Bernoulli(P_SUB) masks
    (jax.random key(42) -> split -> uniform < P_SUB)."""
    if "subs" not in _NC_CACHE:
        b1, b2 = _threefry_pair(0, 42, np.zeros(2, np.uint32),
                                np.arange(2, dtype=np.uint32))
        sub_p = _uniform_lt((b1[0], b2[0]), (T, K, N), P_SUB)
        sub_n = _uniform_lt((b1[1], b2[1]), (T, K, N), P_SUB)
        _NC_CACHE["subs"] = (sub_p, sub_n)
    return _NC_CACHE["subs"]


def _fp8(x):
    return np.ascontiguousarray(np.asarray(x, dtype=np.float32)).astype(NP_FP8)


def _bf16(x):
    return np.ascontiguousarray(np.asarray(x, dtype=np.float32)).astype(NP_BF16)


def build_in_maps(inputs):
    """Host-side prep: returns (in_maps, cnt_p, cnt_n)."""
    return _prep(**{k: v for k, v in inputs.items() if k not in ("t", "n")})


def _prep(vision, belief_features, actions, env_zeros, negative_inds,
          emb, W_ih, W_hh, b_ih, b_hh, W1, b1, W2, b2, **_unused):
    vision = np.asarray(vision, np.float32)
    belief = np.asarray(belief_features, np.float32)
    actions = np.asarray(actions, np.int64)
    env_zeros = np.asarray(env_zeros, np.int64)
    negative_inds = np.asarray(negative_inds, np.int64)
    emb = np.asarray(emb, np.float32)
    W_ih = np.asarray(W_ih, np.float32)
    W_hh = np.asarray(W_hh, np.float32)
    b_ih = np.asarray(b_ih, np.float32)
    b_hh = np.asarray(b_hh, np.float32)
    W1 = np.asarray(W1, np.float32)
    b1v = np.asarray(b1, np.float32)
    W2 = np.asarray(W2, np.float32)
    b2v = np.asarray(b2, np.float32)

    # ---- host-side parameter folding (O(params) only) -----------------
    # G8[a] = x_a @ W_ih.T + b_ih for the 7 actions + zero pad (row 7)
    G8 = np.concatenate([emb, np.zeros((1, A), np.float32)], 0) @ W_ih.T + b_ih
    # gaug32: S*(gi + b_hh) for r (cols :H) and z (cols H:2H)
    gaug32 = S * (G8[:, :2 * H] + b_hh[None, :2 * H])          # (8, 2H)
    gaug_dev = np.zeros((40, H), np.float32)
    gaug_dev[0:8, :] = gaug32[:, :H]
    gaug_dev[32:40, :] = gaug32[:, H:]
    wt = np.ascontiguousarray(
        (S * W_hh).T.reshape(HC, 128, G).transpose(1, 0, 2))   # [128, HC, G]
    w1a = np.ascontiguousarray(
        (S * W1[:, :H]).T.reshape(HC, 128, 32).transpose(1, 0, 2))
    w1q_half = (S * W1[:, H:]).T.reshape(HC, 128, 32).transpose(1, 0, 2)
    w1q = np.ascontiguousarray(
        np.concatenate([w1q_half, w1q_half], axis=2))          # [128, HC, 64]
    # w2s absorbs the 1/S descale of the S-scaled h1; b1d carries S*b1
    w2s = np.concatenate([-W2[0], W2[0]])[:, None] / S         # [64, 1]
    b1d = np.ascontiguousarray(
        S * np.concatenate([b1v, b1v]).reshape(64, 1))
    bhn32 = np.ascontiguousarray(
        (S * b_hh[2 * H:]).reshape(HC, 128).T)                 # [128, HC]
    b2f = float(b2v.reshape(-1)[0])
    b2pat = np.empty((128, 4, K), np.float32)
    b2pat[:, 0:2, :] = -b2f
    b2pat[:, 2:4, :] = b2f

    # ---- masks (host): valid & subsample, weighted --------------------
    sub_p, sub_n = _sub_masks()
    r = np.arange(T + K)[:, None, None]
    c = np.arange(K)[None, :, None]
    z = env_zeros[None, None, :, :]
    zero_hit = np.any((z >= (r - c + 1)[..., None]) & (z <= (r + 1)[..., None]),
                      axis=-1)
    valid_full = (r >= c) & (r < T - 1) & (~zero_hit)          # (T+K, K, N)
    idx = np.arange(T)[:, None] + np.arange(K)[None, :]
    valid = valid_full[idx, np.arange(K)[None, :]]             # (T, K, N)
    mask_p = valid & sub_p
    mask_n = valid & sub_n
    wmask_p = WEIGHT[None, :, None] * mask_p                   # (T, K, N) f32
    wmask_n = WEIGHT[None, :, None] * mask_n
    cnt_p = float(mask_p.sum())
    cnt_n = float(mask_n.sum())

    # ---- per-core inputs ----------------------------------------------
    negatives = vision.reshape(T * N, H)[negative_inds].reshape(T, N, H)

    def chunkT(x):  # (T=B, H) -> [128, HC, B] feature-major chunks
        return np.ascontiguousarray(x.T.reshape(HC, 128, B).transpose(1, 0, 2))

    in_maps = []
    for e in range(NCORES):
        a_pad = np.concatenate([actions[:, e], np.full(K - 1, 7, np.int64)])
        onehot = np.zeros((40, PADW), np.float32)
        onehot[a_pad, np.arange(PADW)] = 1.0
        onehot[a_pad + 32, np.arange(PADW)] = 1.0
        gin_e = np.ascontiguousarray(
            (S * G8[a_pad][:, 2 * H:]).T.reshape(HC, 128, PADW)
            .transpose(1, 0, 2))
        belief_e = chunkT(belief[:, e, :])
        in_maps.append({
            "wt8": _fp8(wt),
            "gaug8": _fp8(gaug_dev),
            "onehot8": _fp8(onehot),
            "gin": _bf16(gin_e),
            "belief8": _fp8(belief_e),
            "beliefb": _bf16(belief_e),
            "vis8": _fp8(chunkT(vision[:, e, :])),
            "neg8": _fp8(chunkT(negatives[:, e, :])),
            "w1a8": _fp8(w1a),
            "w1q8": _fp8(w1q),
            "w2s": _bf16(w2s),
            "b1d": b1d,
            "bhn32": bhn32,
            "b2pat": b2pat,
            "wmask": np.ascontiguousarray(np.concatenate(
                [wmask_p[:, :, e].reshape(BC, 128, K),
                 wmask_n[:, :, e].reshape(BC, 128, K)],
                axis=0).transpose(1, 0, 2)),
        })

    return in_maps, cnt_p, cnt_n


def kernel(**inputs):
    in_maps, cnt_p, cnt_n = build_in_maps(inputs)
    b_hh = np.asarray(inputs["b_hh"], np.float32)
    nc = _build_bass(zero_bhn=not np.any(b_hh[2 * H:]))
    res = run_bass_kernel_spmd(nc, in_maps, core_ids=list(range(NCORES)))
    parts = np.stack([res.results[i]["partials"] for i in range(NCORES)])
    sp_num = float(parts[:, :, 0:2].sum(dtype=np.float64))
    sn_num = float(parts[:, :, 2:4].sum(dtype=np.float64))
    loss = (sp_num / max(cnt_p, 1.0) + sn_num / max(cnt_n, 1.0)) * LOSS_FACTOR
    return np.float32(loss)


# revision 33
# speedup vs baseline: 1.0075x; 1.0075x over previous
"""CPCA-weighted loss kernel for 8 Trainium2 NeuronCores (fp8 pipeline).

Sharding: data-parallel over the env dim n (8 envs -> 1 env/core, params
replicated).  Each core runs the k=16-step GRU over its 256 sequences, the
two-layer classifier for pos/neg logits, softplus + weighted-mask reduce,
and returns per-core partial sums; the host combines them into the scalar
loss.

v2 restructurings vs the bf16 baseline (validated to 7e-5 rel in numpy):
  * All big matmuls in fp8-e4m3 with weights pre-scaled by 32; DoubleRow
    perf mode contracts 256 rows/pass (2x-4x PE throughput).  The 1/32
    descale rides the activation `scale` / stt scalar slots for free.
  * r/z gates keep the rank-8 "onehot" matmul (gi + b_hh into PSUM, fp8);
    the n-gate's b_hh uses the t1 stt scalar slot instead.
  * sigmoid(r) and sigmoid(z) merged into one wide ACT op per half-step
    reading two adjacent PSUM banks.
  * GRU elementwise chain expressed as scalar_tensor_tensor ops (4x DVE
    mode when all operands are bf16 SBUF); h kept in bf16 + fp8 copies
    (fp8 cast on the otherwise-idle Pool/GpSimd engine).
  * pos/neg classifier heads stacked on partitions 0-31/32-63: one
    qp matmul, one pre-stt, one relu per step for both heads.
"""

import numpy as np
import ml_dtypes

import concourse.bass as bass
import concourse.mybir as mybir
import concourse.tile as tile
from concourse import bacc
from concourse.bass_utils import run_bass_kernel_spmd

BF16 = mybir.dt.bfloat16
FP8 = mybir.dt.float8e4
F32 = mybir.dt.float32
AF = mybir.ActivationFunctionType
ALU = mybir.AluOpType
DR = mybir.MatmulPerfMode.DoubleRow

T, N, H, K, A = 256, 8, 512, 16, 4
NUM_ACTIONS = 6
P_SUB = 0.1
LOSS_FACTOR = 0.1
WEIGHT = np.array([5, 4, 3, 3, 2, 2, 2, 2, 1, 1, 1, 1, 1, 1, 1, 1], dtype=np.float32)

NCORES = 8
B = T * N // NCORES            # 256 sequences per core
BC = B // 128                  # 2 partition chunks of the batch
HC = H // 128                  # 4 partition chunks of the hidden dim
G = 3 * H                      # 1536 gate dim
PADW = T + K - 1               # 271 padded action-sequence length
S = 32.0                       # fp8 weight scale
SI = float(1.0 / S)

NP_FP8 = ml_dtypes.float8_e4m3
NP_BF16 = ml_dtypes.bfloat16

_NC_CACHE = {}


def _build_bass(zero_bhn=True):
    """Build the per-core Bass program (identical on all 8 cores)."""
    key = ("nc", zero_bhn)
    if key in _NC_CACHE:
        return _NC_CACHE[key]

    nc = bacc.Bacc("TRN2", target_bir_lowering=False, debug=False)

    # --- DRAM I/O ------------------------------------------------------
    # wt8: [128, HC, G] fp8 = (S*W_hh).T chunked; gate cols 0..511 r,
    # 512..1023 z, 1024..1535 n.
    d_wt = nc.dram_tensor("wt8", [128, HC, G], FP8, kind="ExternalInput")
    # gaug8: [40, H] fp8: rows 0-7 = S*(gi_r + b_hh_r) table (8 actions),
    # rows 32-39 = z ditto.  onehot8: [40, PADW] fp8 one-hot action seq at
    # the same partition strips.
    d_gaug = nc.dram_tensor("gaug8", [40, H], FP8, kind="ExternalInput")
    d_onehot = nc.dram_tensor("onehot8", [40, PADW], FP8, kind="ExternalInput")
    d_gin = nc.dram_tensor("gin", [128, HC, PADW], BF16, kind="ExternalInput")
    d_bel8 = nc.dram_tensor("belief8", [128, HC, B], FP8, kind="ExternalInput")
    d_belb = nc.dram_tensor("beliefb", [128, HC, B], BF16, kind="ExternalInput")
    d_vis8 = nc.dram_tensor("vis8", [128, HC, B], FP8, kind="ExternalInput")
    d_neg8 = nc.dram_tensor("neg8", [128, HC, B], FP8, kind="ExternalInput")
    # w1a8: [128, HC, 32] fp8 (e-part, S*W1a.T); w1q8: [128, HC, 64] fp8
    # (q-part S*W1b.T duplicated to cols 32-63 for the stacked heads)
    d_w1a = nc.dram_tensor("w1a8", [128, HC, 32], FP8, kind="ExternalInput")
    d_w1q = nc.dram_tensor("w1q8", [128, HC, 64], FP8, kind="ExternalInput")
    # w2s: [64, 1] bf16: rows 0-31 = -W2 (pos sign), rows 32-63 = +W2
    d_w2 = nc.dram_tensor("w2s", [64, 1], BF16, kind="ExternalInput")
    d_b1 = nc.dram_tensor("b1d", [64, 1], F32, kind="ExternalInput")
    d_bhn = nc.dram_tensor("bhn32", [128, HC], F32, kind="ExternalInput")
    d_b2pat = nc.dram_tensor("b2pat", [128, 4, K], F32, kind="ExternalInput")
    d_wm = nc.dram_tensor("wmask", [128, 4, K], F32, kind="ExternalInput")
    d_out = nc.dram_tensor("partials", [128, 4], F32, kind="ExternalOutput")

    with tile.TileContext(nc) as tc:
        with (
            tc.tile_pool(name="const", bufs=1) as const,
            tc.tile_pool(name="hbp", bufs=1) as hbp,
            tc.tile_pool(name="h8p", bufs=1) as h8p,
            tc.tile_pool(name="gates", bufs=1) as gates,
            tc.tile_pool(name="cls", bufs=1) as cls,
            tc.tile_pool(name="rzp", bufs=1, space="PSUM") as rzp,
            tc.tile_pool(name="npp", bufs=1, space="PSUM") as npp,
            tc.tile_pool(name="spsum", bufs=1, space="PSUM") as spsum,
            tc.tile_pool(name="lpsum", bufs=1, space="PSUM") as lpsum,
        ):
            # --- constants into SBUF (ordered by first use) -------------
            w1a = const.tile([128, HC, 32], FP8)
            nc.sync.dma_start(w1a[:], d_w1a[:])
            vis = const.tile([128, HC, B], FP8)
            nc.sync.dma_start(vis[:], d_vis8[:])
            neg = const.tile([128, HC, B], FP8)
            nc.gpsimd.dma_start(neg[:], d_neg8[:])
            b1 = const.tile([64, 1], F32)
            nc.sync.dma_start(b1[:], d_b1[:])
            # h kept as per-half tiles so the kt0 matmul wave of step m+1
            # can start as soon as half 0 of h(m) is cast (precise deps)
            h8_0 = [h8p.tile([128, 2, B], FP8, tag=f"h8_{t}", bufs=2,
                             name=f"h8i_{t}") for t in range(2)]
            for t in range(2):
                nc.sync.dma_start(h8_0[t][:], d_bel8[:, 2 * t:2 * t + 2, :])
            gaug = const.tile([40, H], FP8)
            nc.gpsimd.dma_start(gaug[:], d_gaug[:])
            onehot = const.tile([40, PADW], FP8)
            nc.gpsimd.dma_start(onehot[:], d_onehot[:])
            dq = [nc.sync, nc.gpsimd]
            wtj = const.tile([128, HC, G], FP8, name="wt8")
            for j in range(6):
                dq[j % 2].dma_start(
                    wtj[:, :, 256 * j:256 * (j + 1)],
                    d_wt[:, :, 256 * j:256 * (j + 1)])
            hb_0 = [hbp.tile([128, 2, B], BF16, tag=f"hb_{t}", bufs=2,
                             name=f"hbi_{t}") for t in range(2)]
            for t in range(2):
                nc.sync.dma_start(hb_0[t][:], d_belb[:, 2 * t:2 * t + 2, :])
            gin = const.tile([128, HC, PADW], BF16)
            nc.gpsimd.dma_start(gin[:], d_gin[:])
            w1q = const.tile([128, HC, 64], FP8)
            nc.sync.dma_start(w1q[:], d_w1q[:])
            w2 = const.tile([64, 1], BF16)
            nc.sync.dma_start(w2[:], d_w2[:])
            bhn = const.tile([128, HC], F32)
            nc.sync.dma_start(bhn[:], d_bhn[:])
            b2pat = const.tile([128, 4, K], F32)
            nc.gpsimd.dma_start(b2pat[:], d_b2pat[:])
            wm = const.tile([128, 4, K], F32)
            nc.sync.dma_start(wm[:], d_wm[:])

            # --- V1/N1: e-part of classifier, heads stacked [64, B] ----
            # vpad[p, c] = (S*vis[c] @ W1a.T)/S + b1 for c<T, 0 for pads
            ps_vn = spsum.tile([64, B], F32, tag="s", bufs=1, name="ps_vn")
            for kt in range(2):
                nc.tensor.matmul(
                    ps_vn[0:32, :], w1a[:, 2 * kt:2 * kt + 2, :],
                    vis[:, 2 * kt:2 * kt + 2, :],
                    start=(kt == 0), stop=(kt == 1), perf_mode=DR,
                )
            # (DoubleRow + column tile_position fails the ISA check; the
            # neg head runs as 4 plain fp8 matmuls instead)
            for kk in range(HC):
                nc.tensor.matmul(
                    ps_vn[32:64, :], w1a[:, kk, :], neg[:, kk, :],
                    start=(kk == 0), stop=(kk == HC - 1),
                    tile_position=(0, 32),
                )
            # vpad kept at S-scale (descale folded into w2s host-side);
            # b1d is S*b1.  Pad region stays 0 (always masked out).
            vpad = cls.tile([64, PADW + 1], BF16, tag="vpad", name="vpad")
            nc.vector.memset(vpad[:, T:], 0.0)
            nc.scalar.activation(vpad[:, 0:T], ps_vn[:], AF.Copy)
            nc.vector.tensor_scalar_add(vpad[:, 0:T], vpad[:, 0:T],
                                        b1[:, 0:1])

            # logits live in one persistent PSUM bank:
            # [batch-part, (pos0,pos1,neg0,neg1), m], sign folded into w2
            logits = lpsum.tile([128, 4, K], F32, tag="l", bufs=1,
                                name="logits")

            h8 = h8_0
            hb = hb_0
            # classifier state carried one step behind the GRU
            prev_h1 = None
            prev_qp = None

            rcol = lambda j: bass.ts(j, 128)
            zcol = lambda j: bass.ts(HC + j, 128)
            ncol = lambda j: bass.ts(2 * HC + j, 128)

            def emit_logits(ph1, pm):
                for c in range(2):
                    for ch in range(BC):
                        nc.tensor.matmul(
                            logits[:, 2 * c + ch, pm:pm + 1],
                            ph1[32 * c:32 * c + 32, bass.ts(ch, 128)],
                            w2[32 * c:32 * c + 32, 0:1],
                            start=True, stop=True, tile_position=(32 * c, 0),
                        )

            def emit_cls_vector(pqp, pm):
                # pre kept at S-scale; w2s is pre-divided by S host-side.
                # Deprioritized: the scheduler slots these into DVE idle
                # time instead of ahead of the GRU chain ops (the logits
                # have a full step of slack).
                save_prio = tc.cur_priority
                tc.cur_priority = save_prio + 1_000_000
                pre = cls.tile([64, B], BF16, tag="pre", bufs=2,
                               name=f"pre{pm}")
                nc.vector.tensor_add(pre[:], pqp[:],
                                     vpad[:, pm + 1:pm + 1 + B])
                h1 = cls.tile([64, B], BF16, tag="h1", bufs=2,
                              name=f"h1{pm}")
                nc.vector.tensor_scalar_max(h1[:], pre[:], 0.0)
                tc.cur_priority = save_prio
                return h1

            for m in range(K):
                # ---- PE program for this step ------------------------
                # onehot wave (no h dependency), then the kt0 wave (only
                # needs half 0 of h(m-1)), qp of step m-1, the kt1 wave
                # (needs half 1), and finally logits of step m-1.  The
                # early waves keep the PE busy while half 1 of h(m-1) is
                # still being produced by the vector chain.
                # one PSUM tile per (gate, half): dependency tracking is
                # tile-granular, so sigmoid(r) must not share a tile with
                # the z accumulation groups (it would wait for them)
                rp = [rzp.tile([128, 2, B], F32, tag=f"rp{t}", bufs=1,
                               name=f"rp{m}_{t}") for t in range(2)]
                zp = [rzp.tile([128, 2, B], F32, tag=f"zp{t}", bufs=1,
                               name=f"zp{m}_{t}") for t in range(2)]
                nps = [npp.tile([128, 2, B], F32, tag=f"np{t}", bufs=1,
                                name=f"np{m}_{t}") for t in range(2)]
                for t in range(2):
                    for jj in range(2):
                        j = 2 * t + jj
                        nc.tensor.matmul(
                            rp[t][:, jj, :], gaug[0:8, rcol(j)],
                            onehot[0:8, m:m + B],
                            start=True, stop=False, tile_position=(0, 0),
                        )
                        nc.tensor.matmul(
                            zp[t][:, jj, :], gaug[32:40, rcol(j)],
                            onehot[32:40, m:m + B],
                            start=True, stop=False, tile_position=(32, 0),
                        )
                # group-complete order: r(h0), n(h0), r(h1), z(h0),
                # n(h1), z(h1) -- sigmoid(r, h0) and t1(h0) unblock after
                # 4/8 matmuls instead of after the whole 24-matmul train.
                # qp kt0 leads the train: it only needs h half 0 and
                # covers the latency until cast(h1) of the previous step
                # lands (needed by every kt1 matmul).
                def gate_mm(gate, t, jj, kt):
                    j = 2 * t + jj
                    ktile = slice(2 * kt, 2 * kt + 2)
                    if gate == "n":
                        nc.tensor.matmul(
                            nps[t][:, jj, :], wtj[:, ktile, ncol(j)],
                            h8[kt][:],
                            start=(kt == 0), stop=(kt == 1), perf_mode=DR,
                        )
                    else:
                        dst = rp[t] if gate == "r" else zp[t]
                        col = rcol(j) if gate == "r" else zcol(j)
                        nc.tensor.matmul(
                            dst[:, jj, :], wtj[:, ktile, col],
                            h8[kt][:],
                            start=False, stop=(kt == 1), perf_mode=DR,
                        )

                # Interleaved train: kt1 matmuls of a group follow its
                # kt0 pair as soon as possible (kt1 needs cast(h1) of the
                # previous step, which lands ~6 matmuls into the train),
                # so sigmoid(r,h0) fires at matmul #8 instead of #15 and
                # the h0 chain overlaps the rest of the train.
                train = [("r", 0, 0), ("n", 0, 0), ("r", 1, 0),
                         ("r", 0, 1),                       # -> sig(r,h0)
                         ("n", 0, 1),                       # -> t1(h0)
                         ("z", 0, 0), ("z", 0, 1),          # -> sig(z,h0)
                         ("n", 1, 0), ("qp", 0, 0), ("z", 1, 0),
                         ("r", 1, 1),                       # -> sig(r,h1)
                         ("n", 1, 1),                       # -> t1(h1)
                         ("z", 1, 1),                       # -> sig(z,h1)
                         ("qp", 1, 0)]
                for gate, t, kt in train:
                    if gate == "qp":
                        if prev_qp is not None:
                            nc.tensor.matmul(
                                prev_qp[:], w1q[:, 2 * t:2 * t + 2, :],
                                h8[t][:],
                                start=(t == 0), stop=(t == 1), perf_mode=DR,
                            )
                    else:
                        for jj in range(2):
                            gate_mm(gate, t, jj, kt)
                if prev_h1 is not None:
                    emit_logits(*prev_h1)

                # ---- ACT/DVE interleave (emission = producer order;
                # ACT queue lands as sr0, sz0, sr1, tanh0, sz1, tanh1) --
                rzbs = [gates.tile([128, 4, B], BF16, tag="rzb", bufs=2,
                                   name=f"rzb{m}_{t}") for t in range(2)]
                cands = [gates.tile([128, 2, B], BF16, tag="cand", bufs=2,
                                    name=f"c{m}_{t}") for t in range(2)]
                t2s = [gates.tile([128, 2, B], BF16, tag="t2", bufs=2,
                                  name=f"t2{m}_{t}") for t in range(2)]
                t1s = [gates.tile([128, 2, B], BF16, tag="t1", bufs=2,
                                  name=f"t1{m}_{t}") for t in range(2)]

                def emit_t1t2(t):
                    if zero_bhn:
                        # t1 = psum_n * r in one [512] tt (b_hh_n == 0)
                        nc.vector.tensor_mul(t1s[t][:], nps[t][:],
                                             rzbs[t][:, 0:2, :])
                    else:
                        for jj in range(2):
                            j = 2 * t + jj
                            nc.vector.scalar_tensor_tensor(
                                out=t1s[t][:, jj, :], in0=nps[t][:, jj, :],
                                scalar=bhn[:, j:j + 1], op0=ALU.add,
                                in1=rzbs[t][:, jj, :], op1=ALU.mult,
                            )
                    nc.vector.tensor_add(
                        t2s[t][:], t1s[t][:],
                        gin[:, 2 * t:2 * t + 2, m:m + B])

                h8n = [h8p.tile([128, 2, B], FP8, tag=f"h8_{t}", bufs=2,
                                name=f"h8_{m + 1}_{t}") for t in range(2)]
                hbn = [hbp.tile([128, 2, B], BF16, tag=f"hb_{t}", bufs=2,
                                name=f"hb_{m + 1}_{t}") for t in range(2)]

                def emit_tail(t, cast_on_act):
                    # d = h - cand ; e = z*d ; h' = cand + e  (tt, 2x)
                    d = gates.tile([128, 2, B], BF16, tag="d", bufs=2,
                                   name=f"d{m}_{t}")
                    nc.vector.tensor_sub(d[:], hb[t][:], cands[t][:])
                    e = gates.tile([128, 2, B], BF16, tag="e", bufs=2,
                                   name=f"e{m}_{t}")
                    nc.vector.tensor_mul(e[:], rzbs[t][:, 2:4, :], d[:])
                    nc.vector.tensor_add(hbn[t][:], cands[t][:], e[:])
                    if cast_on_act:
                        nc.scalar.activation(h8n[t][:], hbn[t][:], AF.Copy)
                    else:
                        nc.vector.tensor_copy(h8n[t][:], hbn[t][:])

                # Engine-queue orders (priority = emission):
                #   ACT: sr0, sr1, tanh0, sz0, sz1, tanh1, cast1
                #   DVE: t1/t2(h0), t1/t2(h1), tail0+cast0, tail1
                # cast0 stays on DVE (next op in queue after h'0, no
                # cross-engine hop on the critical cycle); cast1 rides
                # the ACT which has slack after tanh1.
                nc.scalar.activation(rzbs[0][:, 0:2, :], rp[0][:],
                                     AF.Sigmoid, scale=SI)
                nc.scalar.activation(rzbs[1][:, 0:2, :], rp[1][:],
                                     AF.Sigmoid, scale=SI)
                emit_t1t2(0)
                nc.scalar.activation(cands[0][:], t2s[0][:], AF.Tanh,
                                     scale=SI)
                emit_t1t2(1)
                nc.scalar.activation(rzbs[0][:, 2:4, :], zp[0][:],
                                     AF.Sigmoid, scale=SI)
                nc.scalar.activation(rzbs[1][:, 2:4, :], zp[1][:],
                                     AF.Sigmoid, scale=SI)
                emit_tail(0, cast_on_act=False)
                nc.scalar.activation(cands[1][:], t2s[1][:], AF.Tanh,
                                     scale=SI)
                emit_tail(1, cast_on_act=True)
                h8 = h8n
                hb = hbn
                # classifier vector work of m-1 last: logits have a full
                # step of slack, the gate chain does not
                if prev_qp is not None:
                    prev_h1 = (emit_cls_vector(prev_qp, m - 1), m - 1)

                # classifier psum for this step (filled next iteration)
                prev_qp = spsum.tile([64, B], F32, tag="s", bufs=1,
                                     name=f"qp{m}")

            # flush the last step's classifier
            nc.tensor.matmul(prev_qp[:], w1q[:, 0:2, :], h8[0][:],
                             start=True, stop=False, perf_mode=DR)
            nc.tensor.matmul(prev_qp[:], w1q[:, 2:4, :], h8[1][:],
                             start=False, stop=True, perf_mode=DR)
            h1f = emit_cls_vector(prev_qp, K - 1)
            emit_logits(h1f, K - 1)
            if prev_h1 is not None:
                emit_logits(*prev_h1)

            # --- softplus + mask-weight reduce ------------------------
            # logits hold s0 = -+(h1 @ W2.T); s = s0 + (-+b2);
            # softplus(s) = max(s,0) + ln(1 + exp(-|s|))
            partials = cls.tile([128, 4, 1], F32, tag="part", name="partials")
            s = cls.tile([128, 4, K], F32, tag="s_aff", name="s_aff")
            nc.vector.tensor_add(s[:], logits[:], b2pat[:])
            rl = cls.tile([128, 4, K], F32, tag="s_rl", name="s_rl")
            nc.vector.tensor_scalar_max(rl[:], s[:], 0.0)
            nab = cls.tile([128, 4, K], F32, tag="s_nab", name="s_nab")
            nc.vector.scalar_tensor_tensor(
                out=nab[:], in0=rl[:], scalar=-2.0, in1=s[:],
                op0=ALU.mult, op1=ALU.add,
            )
            ex = cls.tile([128, 4, K], F32, tag="s_ex", name="s_ex")
            nc.scalar.activation(ex[:], nab[:], AF.Exp)
            lg = cls.tile([128, 4, K], F32, tag="s_lg", name="s_lg")
            nc.scalar.activation(lg[:], ex[:], AF.Ln, bias=1.0)
            sp = cls.tile([128, 4, K], F32, tag="sp", name="sp")
            nc.vector.tensor_add(sp[:], rl[:], lg[:])
            tr = cls.tile([128, 4, K], F32, tag="tr", name="tr")
            nc.vector.tensor_mul(tr[:], sp[:], wm[:])
            nc.vector.reduce_sum(partials[:], tr[:], axis=mybir.AxisListType.X)
            nc.sync.dma_start(d_out[:], partials[:, :, 0])

    nc.compile()
    _NC_CACHE[key] = nc
    return nc


def _threefry_pair(k0, k1, x0, x1):
    """numpy port of jax's threefry2x32 primitive (verified bit-exact)."""
    x0 = x0.astype(np.uint32).copy()
    x1 = x1.astype(np.uint32).copy()
    ks0 = np.uint32(k0)
    ks1 = np.uint32(k1)
    ks2 = np.uint32(ks0 ^ ks1 ^ np.uint32(0x1BD11BDA))

    def rotl(x, d):
        return ((x << np.uint32(d)) | (x >> np.uint32(32 - d))).astype(np.uint32)

    rots = [[13, 15, 26, 6], [17, 29, 16, 24]]
    x0 = (x0 + ks0).astype(np.uint32)
    x1 = (x1 + ks1).astype(np.uint32)
    ks = [ks1, ks2, ks0]
    for i in range(5):
        for r in rots[i % 2]:
            x0 = (x0 + x1).astype(np.uint32)
            x1 = np.uint32(rotl(x1, r) ^ x0)
        x0 = (x0 + ks[i % 3]).astype(np.uint32)
        x1 = (x1 + ks[(i + 1) % 3] + np.uint32(i + 1)).astype(np.uint32)
    return x0, x1


def _uniform_lt(key, shape, thresh):
    """jax.random.uniform(key, shape) < thresh, threefry-partitionable spec."""
    num = int(np.prod(shape))
    b1, b2 = _threefry_pair(key[0], key[1], np.zeros(num, np.uint32),
                            np.arange(num, dtype=np.uint32))
    bits = b1 ^ b2
    fl = ((bits >> np.uint32(9)) | np.uint32(0x3F800000)).view(np.float32) \
        - np.float32(1.0)
    fl = np.maximum(fl, np.float32(0.0))
    return (fl < np.float32(thresh)).reshape(shape)


def _sub_masks():
    """The reference's input-independent Bernoulli(P_SUB) masks
    (jax.random key(42) -> split -> uniform < P_SUB)."""
    if "subs" not in _NC_CACHE:
        b1, b2 = _threefry_pair(0, 42, np.zeros(2, np.uint32),
                                np.arange(2, dtype=np.uint32))
        sub_p = _uniform_lt((b1[0], b2[0]), (T, K, N), P_SUB)
        sub_n = _uniform_lt((b1[1], b2[1]), (T, K, N), P_SUB)
        _NC_CACHE["subs"] = (sub_p, sub_n)
    return _NC_CACHE["subs"]


def _fp8(x):
    return np.ascontiguousarray(np.asarray(x, dtype=np.float32)).astype(NP_FP8)


def _bf16(x):
    return np.ascontiguousarray(np.asarray(x, dtype=np.float32)).astype(NP_BF16)


def build_in_maps(inputs):
    """Host-side prep: returns (in_maps, cnt_p, cnt_n)."""
    return _prep(**{k: v for k, v in inputs.items() if k not in ("t", "n")})


def _prep(vision, belief_features, actions, env_zeros, negative_inds,
          emb, W_ih, W_hh, b_ih, b_hh, W1, b1, W2, b2, **_unused):
    vision = np.asarray(vision, np.float32)
    belief = np.asarray(belief_features, np.float32)
    actions = np.asarray(actions, np.int64)
    env_zeros = np.asarray(env_zeros, np.int64)
    negative_inds = np.asarray(negative_inds, np.int64)
    emb = np.asarray(emb, np.float32)
    W_ih = np.asarray(W_ih, np.float32)
    W_hh = np.asarray(W_hh, np.float32)
    b_ih = np.asarray(b_ih, np.float32)
    b_hh = np.asarray(b_hh, np.float32)
    W1 = np.asarray(W1, np.float32)
    b1v = np.asarray(b1, np.float32)
    W2 = np.asarray(W2, np.float32)
    b2v = np.asarray(b2, np.float32)

    # ---- host-side parameter folding (O(params) only) -----------------
    # G8[a] = x_a @ W_ih.T + b_ih for the 7 actions + zero pad (row 7)
    G8 = np.concatenate([emb, np.zeros((1, A), np.float32)], 0) @ W_ih.T + b_ih
    # gaug32: S*(gi + b_hh) for r (cols :H) and z (cols H:2H)
    gaug32 = S * (G8[:, :2 * H] + b_hh[None, :2 * H])          # (8, 2H)
    gaug_dev = np.zeros((40, H), np.float32)
    gaug_dev[0:8, :] = gaug32[:, :H]
    gaug_dev[32:40, :] = gaug32[:, H:]
    wt = np.ascontiguousarray(
        (S * W_hh).T.reshape(HC, 128, G).transpose(1, 0, 2))   # [128, HC, G]
    w1a = np.ascontiguousarray(
        (S * W1[:, :H]).T.reshape(HC, 128, 32).transpose(1, 0, 2))
    w1q_half = (S * W1[:, H:]).T.reshape(HC, 128, 32).transpose(1, 0, 2)
    w1q = np.ascontiguousarray(
        np.concatenate([w1q_half, w1q_half], axis=2))          # [128, HC, 64]
    # w2s absorbs the 1/S descale of the S-scaled h1; b1d carries S*b1
    w2s = np.concatenate([-W2[0], W2[0]])[:, None] / S         # [64, 1]
    b1d = np.ascontiguousarray(
        S * np.concatenate([b1v, b1v]).reshape(64, 1))
    bhn32 = np.ascontiguousarray(
        (S * b_hh[2 * H:]).reshape(HC, 128).T)                 # [128, HC]
    b2f = float(b2v.reshape(-1)[0])
    b2pat = np.empty((128, 4, K), np.float32)
    b2pat[:, 0:2, :] = -b2f
    b2pat[:, 2:4, :] = b2f

    # ---- masks (host): valid & subsample, weighted --------------------
    sub_p, sub_n = _sub_masks()
    r = np.arange(T + K)[:, None, None]
    c = np.arange(K)[None, :, None]
    z = env_zeros[None, None, :, :]
    zero_hit = np.any((z >= (r - c + 1)[..., None]) & (z <= (r + 1)[..., None]),
                      axis=-1)
    valid_full = (r >= c) & (r < T - 1) & (~zero_hit)          # (T+K, K, N)
    idx = np.arange(T)[:, None] + np.arange(K)[None, :]
    valid = valid_full[idx, np.arange(K)[None, :]]             # (T, K, N)
    mask_p = valid & sub_p
    mask_n = valid & sub_n
    wmask_p = WEIGHT[None, :, None] * mask_p                   # (T, K, N) f32
    wmask_n = WEIGHT[None, :, None] * mask_n
    cnt_p = float(mask_p.sum())
    cnt_n = float(mask_n.sum())

    # ---- per-core inputs ----------------------------------------------
    negatives = vision.reshape(T * N, H)[negative_inds].reshape(T, N, H)

    def chunkT(x):  # (T=B, H) -> [128, HC, B] feature-major chunks
        return np.ascontiguousarray(x.T.reshape(HC, 128, B).transpose(1, 0, 2))

    in_maps = []
    for e in range(NCORES):
        a_pad = np.concatenate([actions[:, e], np.full(K - 1, 7, np.int64)])
        onehot = np.zeros((40, PADW), np.float32)
        onehot[a_pad, np.arange(PADW)] = 1.0
        onehot[a_pad + 32, np.arange(PADW)] = 1.0
        gin_e = np.ascontiguousarray(
            (S * G8[a_pad][:, 2 * H:]).T.reshape(HC, 128, PADW)
            .transpose(1, 0, 2))
        belief_e = chunkT(belief[:, e, :])
        in_maps.append({
            "wt8": _fp8(wt),
            "gaug8": _fp8(gaug_dev),
            "onehot8": _fp8(onehot),
            "gin": _bf16(gin_e),
            "belief8": _fp8(belief_e),
            "beliefb": _bf16(belief_e),
            "vis8": _fp8(chunkT(vision[:, e, :])),
            "neg8": _fp8(chunkT(negatives[:, e, :])),
            "w1a8": _fp8(w1a),
            "w1q8": _fp8(w1q),
            "w2s": _bf16(w2s),
            "b1d": b1d,
            "bhn32": bhn32,
            "b2pat": b2pat,
            "wmask": np.ascontiguousarray(np.concatenate(
                [wmask_p[:, :, e].reshape(BC, 128, K),
                 wmask_n[:, :, e].reshape(BC, 128, K)],
                axis=0).transpose(1, 0, 2)),
        })

    return in_maps, cnt_p, cnt_n


def kernel(**inputs):
    in_maps, cnt_p, cnt_n = build_in_maps(inputs)
    b_hh = np.asarray(inputs["b_hh"], np.float32)
    nc = _build_bass(zero_bhn=not np.any(b_hh[2 * H:]))
    res = run_bass_kernel_spmd(nc, in_maps, core_ids=list(range(NCORES)))
    parts = np.stack([res.results[i]["partials"] for i in range(NCORES)])
    sp_num = float(parts[:, :, 0:2].sum(dtype=np.float64))
    sn_num = float(parts[:, :, 2:4].sum(dtype=np.float64))
    loss = (sp_num / max(cnt_p, 1.0) + sn_num / max(cnt_n, 1.0)) * LOSS_FACTOR
    return np.float32(loss)
